# revision 18
# baseline (speedup 1.0000x reference)
"""ChannelBlock kernel for Trainium2 — 8 NeuronCores, data-parallel over batch.

Per batch elem (C=384, N=784=28x28, 8 heads, hd=48, mlp=1536):
  x1  = x + dwconv3x3(x, cpe0)
  x2  = x1 + proj(chan_attn(LN(x1))) + proj_b
  x3  = x2 + dwconv3x3(x2, cpe1)
  out = x3 + fc2(gelu(fc1(LN(x3)) + b1)) + b2

Device: channel-major carrier [C, N] (no spatial padding; conv boundary
handled by clipped access patterns), float32r on the carrier path (conv diag
matmuls, LN stats), bf16 on transformer matmuls.
kernel(**inputs) accepts the full unsharded inputs and returns full output.
"""

import contextlib
import os
import numpy as np
import ml_dtypes

import concourse.bacc as bacc
import concourse.bass as bass
from concourse import mybir, bass_utils
from concourse.tile import TileContext, add_dep_helper

F32 = mybir.dt.float32
F32R = mybir.dt.float32r
BF16 = mybir.dt.bfloat16
AF = mybir.ActivationFunctionType
ALU = mybir.AluOpType

B = 32
NCORES = 8
BE = B // NCORES
C = 384
H = W = 28
N = H * W
PW = W + 2            # 30
PH = H + 2            # 30
PN = PH * PW          # 900
HEADS = 8
HD = C // HEADS       # 48
MLP = 4 * C
EPS = 1e-5
CT = C // 128         # 3
MT = MLP // 128       # 12
CH = 2                # token chunks (of 14 spatial rows)
CHR = 14              # rows per chunk
CHW = N // CH         # 392
TOKT = 7
TOKW = N // TOKT      # 112

_CACHE = {}


def _iview(t, ch, dh=0, dw=0):
    """Interior chunk view of padded [128, PH, PW] tile at spatial shift."""
    h0 = CHR * ch + 1 + dh
    return t[:, h0:h0 + CHR, 1 + dw:29 + dw]


def _row_pieces(lo, hi):
    out = []
    t = lo // 128
    while lo < hi:
        top = min(hi, (t + 1) * 128)
        out.append((t, lo, top))
        lo = top
        t += 1
    return out


def build_module():
    nc = bacc.Bacc("TRN2", target_bir_lowering=False, debug=False,
                   num_devices=NCORES)
    xin = nc.dram_tensor("xin", [BE, C, PN], F32R, kind="ExternalInput")
    outx = nc.dram_tensor("outx", [BE, C, N], F32, kind="ExternalOutput")
    diags = nc.dram_tensor("diags", [128, 2 * 9 * CT * 128], F32R,
                           kind="ExternalInput")
    wq = nc.dram_tensor("wq", [128, CT * C], BF16, kind="ExternalInput")
    wkv = nc.dram_tensor("wkv", [128, CT * 2 * C], BF16, kind="ExternalInput")
    wp = nc.dram_tensor("wp", [128, CT * C], BF16, kind="ExternalInput")
    w1 = nc.dram_tensor("w1", [128, CT * MLP], BF16, kind="ExternalInput")
    w2 = nc.dram_tensor("w2", [128, MT * C], BF16, kind="ExternalInput")
    pb = nc.dram_tensor("pb", [128, CT], F32, kind="ExternalInput")
    b1 = nc.dram_tensor("b1", [128, MT], F32, kind="ExternalInput")
    b2 = nc.dram_tensor("b2", [128, CT], F32, kind="ExternalInput")
    ident = nc.dram_tensor("ident", [HD, HD], BF16, kind="ExternalInput")
    zpad = nc.dram_tensor("zpad", [128, PN], F32R, kind="ExternalInput")

    with TileContext(nc) as tc:
        _emit(nc, tc, xin, outx, diags, wq, wkv, wp, w1, w2, pb, b1, b2, ident, zpad)
    nc.compile()
    return nc


def _emit(nc, tc, xin, outx, diags, wq, wkv, wp, w1, w2, pb, b1, b2, ident, zpad):
    with contextlib.ExitStack() as ctx:
        consts = ctx.enter_context(tc.tile_pool(name="consts", bufs=1))
        carr = ctx.enter_context(tc.tile_pool(name="carr", bufs=2))
        carr1 = ctx.enter_context(tc.tile_pool(name="carr1", bufs=1))
        acts = ctx.enter_context(tc.tile_pool(name="acts", bufs=1))
        xbp = ctx.enter_context(tc.tile_pool(name="xbp", bufs=2))
        smalls = ctx.enter_context(tc.tile_pool(name="smalls", bufs=4))
        mlpp = ctx.enter_context(tc.tile_pool(name="mlpp", bufs=2))
        ps_big = ctx.enter_context(tc.tile_pool(name="ps_big", bufs=3,
                                                space="PSUM"))
        ps_row = ctx.enter_context(tc.tile_pool(name="ps_row", bufs=1,
                                                space="PSUM"))
        ps_sm = ctx.enter_context(tc.tile_pool(name="ps_sm", bufs=1,
                                               space="PSUM"))

        # ---- resident constants ----
        sdg = consts.tile([128, 2 * 9 * CT * 128], F32R)
        swq = consts.tile([128, CT * C], BF16)
        swkv = consts.tile([128, CT * 2 * C], BF16)
        swp = consts.tile([128, CT * C], BF16)
        sw1 = consts.tile([128, CT * MLP], BF16)
        sw2 = consts.tile([128, MT * C], BF16)
        spb = consts.tile([128, CT], F32)
        sb1 = consts.tile([128, MT], F32)
        sb2 = consts.tile([128, CT], F32)
        sident = consts.tile([HD, HD], BF16)
        zt = consts.tile([128, PN], F32R)
        nc.sync.dma_start(out=zt[:], in_=zpad[:])
        for dst, src in ((sdg, diags), (swq, wq), (swkv, wkv), (swp, wp),
                         (sw1, w1), (sw2, w2), (spb, pb), (sb1, b1),
                         (sb2, b2), (sident, ident)):
            nc.sync.dma_start(out=dst[:], in_=src[:])
        ones_r = consts.tile([128, 1], F32R)
        ones_f = consts.tile([128, 1], F32)
        ones_b = consts.tile([128, 1], BF16)
        eps_t = consts.tile([1, 1], F32)
        nc.vector.memset(ones_f[:], 1.0)
        nc.vector.tensor_copy(ones_r[:], ones_f[:])  # f32r memset is illegal
        nc.vector.memset(ones_b[:], 1.0)
        nc.vector.memset(eps_t[:], EPS)

        def dgv(cv, j, ct):
            b = (cv * 9 + j) * CT + ct
            return sdg[:, b * 128:(b + 1) * 128]

        def conv(src_pad, cv, dst):
            """dst = src + dwconv3x3(src); center tap has +1 folded in.
            src_pad: [CT][128, PH, PW] zero-padded; dst: [CT][128, N]."""
            for ct in range(CT):
                for ch in range(CH):
                    ps = ps_big.tile([128, CHW], F32, tag="ps", name="ps")
                    j = 0
                    for dh in (-1, 0, 1):
                        for dw in (-1, 0, 1):
                            nc.tensor.matmul(
                                ps[:], dgv(cv, j, ct),
                                _iview(src_pad[ct], ch, dh, dw),
                                start=(j == 0), stop=(j == 8))
                            j += 1
                    nc.scalar.copy(out=dst[ct][:, ch * CHW:(ch + 1) * CHW],
                                   in_=ps[:])

        def layernorm(src, xn, acc):
            """xn (bf16) = (src - mean_c) * rsqrt(var_c + eps); src [CT][128,N]."""
            sq = [acts.tile([128, N], BF16, tag=f"sq{t}", name=f"sq{t}")
                  for t in range(CT)]
            for t in range(CT):
                nc.scalar.activation(out=sq[t][:], in_=src[t][:],
                                     func=AF.Square)
            arow = smalls.tile([1, N], BF16, tag="arow", name="arow")
            brow = smalls.tile([1, N], BF16, tag="brow", name="brow")
            for ch in range(CH):
                sl = slice(ch * CHW, (ch + 1) * CHW)
                ps_s = ps_row.tile([1, CHW], F32, tag="psum_s", name="psum_s")
                ps_q = ps_row.tile([1, CHW], F32, tag="psum_q", name="psum_q")
                for t in range(CT):
                    nc.tensor.matmul(ps_s[:], ones_r[:], src[t][:, sl],
                                     start=(t == 0), stop=(t == CT - 1))
                for t in range(CT):
                    nc.tensor.matmul(ps_q[:], ones_b[:], sq[t][:, sl],
                                     start=(t == 0), stop=(t == CT - 1))
                u = smalls.tile([1, CHW], F32, tag="u", name="u")
                nc.scalar.activation(out=u[:], in_=ps_s[:], func=AF.Square,
                                     scale=float(1.0 / C))          # mean^2
                v = smalls.tile([1, CHW], F32, tag="v", name="v")
                nc.vector.scalar_tensor_tensor(
                    out=v[:], in0=ps_q[:], scalar=float(1.0 / C), in1=u[:],
                    op0=ALU.mult, op1=ALU.subtract)                 # var
                lv = smalls.tile([1, CHW], F32, tag="lv", name="lv")
                i_ln = nc.scalar.activation(out=lv[:], in_=v[:], func=AF.Ln,
                                            bias=eps_t[:])
                acc.append(i_ln)
                acc.append(nc.scalar.activation(
                    out=arow[:, sl], in_=lv[:], func=AF.Exp, scale=-0.5))
                nc.vector.scalar_tensor_tensor(
                    out=brow[:, sl], in0=ps_s[:], scalar=float(-1.0 / C),
                    in1=arow[:, sl], op0=ALU.mult, op1=ALU.mult)    # -mean*rstd
            ab = acts.tile([128, N], BF16, tag="ab", name="ab")
            bb = acts.tile([128, N], BF16, tag="bb", name="bb")
            nc.gpsimd.partition_broadcast(ab[:], arow[:])
            nc.gpsimd.partition_broadcast(bb[:], brow[:])
            for t in range(CT):
                t1 = xbp.tile([128, N], BF16, tag="lnt1", name="lnt1")
                nc.vector.tensor_tensor(out=t1[:], in0=src[t][:], in1=ab[:],
                                        op=ALU.mult)
                nc.vector.tensor_tensor(out=xn[t][:], in0=t1[:], in1=bb[:],
                                        op=ALU.add)

        def attention(xn, x2, carrier1, acc):
            # q channel-major [CT][128, N]
            q = [acts.tile([128, N], BF16, tag=f"q{m}", name=f"q{m}")
                 for m in range(CT)]
            for m in range(CT):
                for ch in range(CH):
                    sl = slice(ch * CHW, (ch + 1) * CHW)
                    ps = ps_big.tile([128, CHW], F32, tag="ps", name="ps")
                    for t in range(CT):
                        nc.tensor.matmul(
                            ps[:], swq[:, t * C + m * 128: t * C + (m + 1) * 128],
                            xn[t][:, sl], start=(t == 0), stop=(t == CT - 1))
                    nc.vector.tensor_copy(q[m][:, sl], ps[:])
            # k|v token-major [TOKT][112, 768]
            kv = [acts.tile([TOKW, 2 * C], BF16, tag=f"kv{i}", name=f"kv{i}")
                  for i in range(TOKT)]
            for i in range(TOKT):
                tok = slice(i * TOKW, (i + 1) * TOKW)
                for nch in range(2):
                    ps = ps_big.tile([TOKW, C], F32, tag="ps", name="ps")
                    for t in range(CT):
                        nc.tensor.matmul(
                            ps[:], xn[t][:, tok],
                            swkv[:, t * 2 * C + nch * C: t * 2 * C + (nch + 1) * C],
                            start=(t == 0), stop=(t == CT - 1))
                    nc.vector.tensor_copy(kv[i][:, nch * C:(nch + 1) * C],
                                          ps[:])
            # per-head gram -> exp(+rowsum) -> transpose -> block-diag ET
            etb = [acts.tile([128, C], BF16, tag=f"etb{t}", name=f"etb{t}")
                   for t in range(CT)]
            rz = [smalls.tile([128, 1], F32, tag=f"rz{t}", name=f"rz{t}")
                  for t in range(CT)]
            for t in range(CT):
                nc.vector.memset(etb[t][:], 0.0)
            for h in range(HEADS):
                psg = ps_sm.tile([HD, HD], F32, tag="gram", name="gram")
                for i in range(TOKT):
                    nc.tensor.matmul(
                        psg[:], kv[i][:, h * HD:(h + 1) * HD],
                        kv[i][:, C + h * HD:C + (h + 1) * HD],
                        start=(i == 0), stop=(i == TOKT - 1))
                e_s = smalls.tile([HD, HD], BF16, tag="e_s", name="e_s")
                z = smalls.tile([HD, 1], F32, tag="z", name="z")
                acc.append(nc.scalar.activation(out=e_s[:], in_=psg[:],
                                                func=AF.Exp, accum_out=z[:]))
                pst = ps_sm.tile([HD, HD], BF16, tag="et", name="et")
                nc.tensor.transpose(pst[:], e_s[:], sident[:])
                ets = smalls.tile([HD, HD], BF16, tag="ets", name="ets")
                nc.vector.tensor_copy(ets[:], pst[:])
                zr = smalls.tile([HD, 1], F32, tag="zr", name="zr")
                nc.vector.reciprocal(zr[:], z[:])
                r0 = h * HD
                # scatter via DMA (SWDGE): compute-op SBUF APs must be
                # 32-aligned in start partition; DMA is exempt.
                for (tt, lo, hi) in _row_pieces(r0, r0 + HD):
                    nc.gpsimd.dma_start(
                        out=etb[tt][lo - 128 * tt:hi - 128 * tt, r0:r0 + HD],
                        in_=ets[lo - r0:hi - r0, :])
                    nc.gpsimd.dma_start(
                        out=rz[tt][lo - 128 * tt:hi - 128 * tt, :],
                        in_=zr[lo - r0:hi - r0, :])
            # attn out (unnormalized) then 1/Z in the copy
            ao = [acts.tile([128, N], BF16, tag=f"ao{m}", name=f"ao{m}")
                  for m in range(CT)]
            for m in range(CT):
                for ch in range(CH):
                    sl = slice(ch * CHW, (ch + 1) * CHW)
                    ps = ps_big.tile([128, CHW], F32, tag="ps", name="ps")
                    for t in range(CT):
                        nc.tensor.matmul(
                            ps[:], etb[t][:, m * 128:(m + 1) * 128],
                            q[t][:, sl], start=(t == 0), stop=(t == CT - 1))
                    nc.vector.tensor_scalar_mul(ao[m][:, sl], ps[:], rz[m][:])
            # proj + bias + residual -> x2
            for m in range(CT):
                for ch in range(CH):
                    sl = slice(ch * CHW, (ch + 1) * CHW)
                    ps = ps_big.tile([128, CHW], F32, tag="ps", name="ps")
                    for t in range(CT):
                        nc.tensor.matmul(
                            ps[:], swp[:, t * C + m * 128: t * C + (m + 1) * 128],
                            ao[t][:, sl], start=(t == 0), stop=(t == CT - 1))
                    nc.vector.scalar_tensor_tensor(
                        out=_iview(x2[m], ch),
                        in0=ps[:].rearrange("p (a b) -> p a b", a=CHR),
                        scalar=spb[:, m:m + 1],
                        in1=carrier1[m][:, sl].rearrange("p (a b) -> p a b",
                                                         a=CHR),
                        op0=ALU.add, op1=ALU.add)

        def mlp(xn2, carrier2, ot, acc):
            for ch in range(CH):
                sl = slice(ch * CHW, (ch + 1) * CHW)
                h1 = [mlpp.tile([128, CHW], BF16, tag=f"h1_{m}", name=f"h1_{m}")
                      for m in range(MT)]
                for m in range(MT):
                    ps = ps_big.tile([128, CHW], F32, tag="ps", name="ps")
                    for t in range(CT):
                        nc.tensor.matmul(
                            ps[:],
                            sw1[:, t * MLP + m * 128: t * MLP + (m + 1) * 128],
                            xn2[t][:, sl], start=(t == 0), stop=(t == CT - 1))
                    acc.append(nc.scalar.activation(
                        out=h1[m][:], in_=ps[:], func=AF.Gelu,
                        bias=sb1[:, m:m + 1]))
                for m in range(CT):
                    ps = ps_big.tile([128, CHW], F32, tag="ps", name="ps")
                    for kt in range(MT):
                        nc.tensor.matmul(
                            ps[:], sw2[:, kt * C + m * 128: kt * C + (m + 1) * 128],
                            h1[kt][:], start=(kt == 0), stop=(kt == MT - 1))
                    nc.vector.scalar_tensor_tensor(
                        out=ot[m][:, sl], in0=ps[:], scalar=sb2[:, m:m + 1],
                        in1=carrier2[m][:, sl], op0=ALU.add, op1=ALU.add)

        prev_gelus = None
        for e in range(BE):
            c0 = [carr.tile([128, PH, PW], F32R, tag=f"pad{t}", name=f"pad{t}")
                  for t in range(CT)]
            for t in range(CT):
                nc.sync.dma_start(
                    out=c0[t][:],
                    in_=xin[e, t * 128:(t + 1) * 128, :].rearrange(
                        "p (a b) -> p a b", a=PH))
            carrier1 = [carr1.tile([128, N], F32R, tag=f"c1_{t}",
                                   name=f"c1_{t}") for t in range(CT)]
            conv(c0, 0, carrier1)
            # ACT table-set ordering: all exp/ln of this elem come after the
            # previous elem's gelus (keeps set switches at 2 per element).
            a_ops = []
            g_ops = []
            xn1 = [acts.tile([128, N], BF16, tag=f"xn1_{t}", name=f"xn1_{t}")
                   for t in range(CT)]
            layernorm(carrier1, xn1, a_ops)
            x2 = [carr.tile([128, PH, PW], F32R, tag=f"pad{t}", name=f"pad{t}")
                  for t in range(CT)]
            for t in range(CT):
                nc.vector.tensor_copy(x2[t][:], zt[:].rearrange(
                    "p (a b) -> p a b", a=PH))
            attention(xn1, x2, carrier1, a_ops)
            carrier2 = [carr1.tile([128, N], F32R, tag=f"c2_{t}",
                                   name=f"c2_{t}") for t in range(CT)]
            conv(x2, 1, carrier2)
            xn2 = [acts.tile([128, N], BF16, tag=f"xn2_{t}", name=f"xn2_{t}")
                   for t in range(CT)]
            layernorm(carrier2, xn2, a_ops)
            ot = [carr1.tile([128, N], F32, tag=f"ot{t}", name=f"ot{t}")
                  for t in range(CT)]
            mlp(xn2, carrier2, ot, g_ops)
            if prev_gelus and int(os.environ.get('ACT_ORDER', '0')):
                # every gelu (set B) of the previous elem precedes every
                # exp/ln (set A) of this elem -> 2 table switches per elem
                for g in prev_gelus:
                    add_dep_helper(g.ins, a_ops[0].ins, sync=True,
                                   reason="ACT set ordering")
                for op in a_ops[1:]:
                    add_dep_helper(a_ops[0].ins, op.ins, sync=True,
                                   reason="ACT set ordering")
            prev_gelus = g_ops
            for t in range(CT):
                nc.sync.dma_start(out=outx[e, t * 128:(t + 1) * 128, :],
                                  in_=ot[t][:])


# ======================= host side =======================

def _tobf(a):
    return np.asarray(a, np.float32).astype(ml_dtypes.bfloat16)


def _prep_weights(inputs):
    qkv_w = np.asarray(inputs["qkv_w"], np.float32)
    g1 = np.asarray(inputs["norm1_g"], np.float32)
    g2 = np.asarray(inputs["norm2_g"], np.float32)
    assert np.allclose(np.asarray(inputs["norm1_b"], np.float32), 0)
    assert np.allclose(np.asarray(inputs["norm2_b"], np.float32), 0)
    assert np.allclose(np.asarray(inputs["cpe0_b"], np.float32), 0)
    assert np.allclose(np.asarray(inputs["cpe1_b"], np.float32), 0)

    scale = HD ** (-0.5)
    wq_t = (qkv_w[0:C] * g1[None, :]).T.copy()              # [in C, out C]
    wk_t = (qkv_w[C:2 * C] * scale * g1[None, :]).T.copy()
    wv_t = (qkv_w[2 * C:3 * C] * g1[None, :]).T.copy()
    wkv_t = np.concatenate([wk_t, wv_t], axis=1)            # [C, 2C]
    wp_t = np.asarray(inputs["proj_w"], np.float32).T.copy()
    w1_t = (np.asarray(inputs["fc1_w"], np.float32) * g2[None, :]).T.copy()
    w2_t = np.asarray(inputs["fc2_w"], np.float32).T.copy()

    def kpack(wt):  # [Cin, F] -> [128, (Cin/128) * F] K-tile blocks
        kin, f = wt.shape
        nt = kin // 128
        return np.concatenate(
            [wt[t * 128:(t + 1) * 128, :] for t in range(nt)], axis=1).copy()

    def diag_pack():
        d = np.zeros((128, 2 * 9 * CT * 128), np.float32)
        for cv, wname in ((0, "cpe0_w"), (1, "cpe1_w")):
            w = np.asarray(inputs[wname], np.float32).reshape(C, 9)
            for j in range(9):
                wj = w[:, j].copy()
                if j == 4:
                    wj = wj + 1.0
                for t in range(CT):
                    blk = (cv * 9 + j) * CT + t
                    dd = d[:, blk * 128:(blk + 1) * 128]
                    np.fill_diagonal(dd, wj[t * 128:(t + 1) * 128])
        return d

    def colpack(b, nt):  # [nt*128] -> [128, nt]
        return np.asarray(b, np.float32).reshape(nt, 128).T.copy()

    return {
        "diags": diag_pack(),
        "wq": _tobf(kpack(wq_t)), "wkv": _tobf(kpack(wkv_t)),
        "wp": _tobf(kpack(wp_t)), "w1": _tobf(kpack(w1_t)),
        "w2": _tobf(kpack(w2_t)),
        "pb": colpack(inputs["proj_b"], CT),
        "b1": colpack(inputs["fc1_b"], MT),
        "b2": colpack(inputs["fc2_b"], CT),
        "ident": _tobf(np.eye(HD, dtype=np.float32)),
        "zpad": np.zeros((128, PN), np.float32),
    }


def _pad_x(xe):
    """[BE, N, C] -> channel-major zero-padded [BE, C, PN] float32."""
    xp = np.zeros((xe.shape[0], C, PH, PW), np.float32)
    xc = np.transpose(np.asarray(xe, np.float32), (0, 2, 1))
    xp[:, :, 1:H + 1, 1:W + 1] = xc.reshape(-1, C, H, W)
    return xp.reshape(-1, C, PN)


def kernel(**inputs):
    x = np.asarray(inputs["x"], np.float32)
    if "nc" not in _CACHE:
        _CACHE["nc"] = build_module()
    nc = _CACHE["nc"]
    wmap = _prep_weights(inputs)

    in_maps = []
    for core in range(NCORES):
        xe = x[core * BE:(core + 1) * BE]                   # [BE, N, C]
        m = {"xin": _pad_x(xe)}
        m.update(wmap)
        in_maps.append(m)

    trace = bool(int(os.environ.get("TRN_KERNEL_TRACE", "0")))
    res = bass_utils.run_bass_kernel_spmd(
        nc, in_maps, core_ids=list(range(NCORES)), trace=trace)
    if trace:
        _CACHE["exec_time_ns"] = res.exec_time_ns

    outp = np.empty((B, N, C), np.float32)
    for core in range(NCORES):
        oc = res.results[core]["outx"]
        outp[core * BE:(core + 1) * BE] = np.transpose(oc, (0, 2, 1))
    return outp


# revision 19
# speedup vs baseline: 1.1833x; 1.1833x over previous
"""ChannelBlock kernel for Trainium2 — 8 NeuronCores, data-parallel over batch.

Per batch elem (C=384, N=784=28x28, 8 heads, hd=48, mlp=1536):
  x1  = x + dwconv3x3(x, cpe0)
  x2  = x1 + proj(chan_attn(LN(x1))) + proj_b
  x3  = x2 + dwconv3x3(x2, cpe1)
  out = x3 + fc2(gelu(fc1(LN(x3)) + b1)) + b2

Device: channel-major carrier [C, N] (no spatial padding; conv boundary
handled by clipped access patterns), float32r on the carrier path (conv diag
matmuls, LN stats), bf16 on transformer matmuls.
kernel(**inputs) accepts the full unsharded inputs and returns full output.
"""

import contextlib
import os
import numpy as np
import ml_dtypes

import concourse.bacc as bacc
import concourse.bass as bass
from concourse import mybir, bass_utils
from concourse.tile import TileContext, add_dep_helper

F32 = mybir.dt.float32
F32R = mybir.dt.float32r
BF16 = mybir.dt.bfloat16
AF = mybir.ActivationFunctionType
ALU = mybir.AluOpType

B = 32
NCORES = 8
BE = B // NCORES
C = 384
H = W = 28
N = H * W
PW = W + 2            # 30
PH = H + 2            # 30
PN = PH * PW          # 900
HEADS = 8
HD = C // HEADS       # 48
MLP = 4 * C
EPS = 1e-5
CT = C // 128         # 3
MT = MLP // 128       # 12
CH = 2                # token chunks (of 14 spatial rows)
CHR = 14              # rows per chunk
CHW = N // CH         # 392
TOKT = 7
TOKW = N // TOKT      # 112

_CACHE = {}


def _iview(t, ch, dh=0, dw=0):
    """Interior chunk view of padded [128, PH, PW] tile at spatial shift."""
    h0 = CHR * ch + 1 + dh
    return t[:, h0:h0 + CHR, 1 + dw:29 + dw]


def _row_pieces(lo, hi):
    out = []
    t = lo // 128
    while lo < hi:
        top = min(hi, (t + 1) * 128)
        out.append((t, lo, top))
        lo = top
        t += 1
    return out


def build_module():
    nc = bacc.Bacc("TRN2", target_bir_lowering=False, debug=False,
                   num_devices=NCORES)
    xin = nc.dram_tensor("xin", [BE, C, PN], F32R, kind="ExternalInput")
    outx = nc.dram_tensor("outx", [BE, C, N], F32, kind="ExternalOutput")
    diags = nc.dram_tensor("diags", [128, 2 * 9 * CT * 128], F32R,
                           kind="ExternalInput")
    wq = nc.dram_tensor("wq", [128, CT * C], BF16, kind="ExternalInput")
    wkv = nc.dram_tensor("wkv", [128, CT * 2 * C], BF16, kind="ExternalInput")
    wp = nc.dram_tensor("wp", [128, CT * C], BF16, kind="ExternalInput")
    w1 = nc.dram_tensor("w1", [128, CT * MLP], BF16, kind="ExternalInput")
    w2 = nc.dram_tensor("w2", [128, MT * C], BF16, kind="ExternalInput")
    pb = nc.dram_tensor("pb", [128, CT], F32, kind="ExternalInput")
    b1 = nc.dram_tensor("b1", [128, MT], F32, kind="ExternalInput")
    b2 = nc.dram_tensor("b2", [128, CT], F32, kind="ExternalInput")
    ident = nc.dram_tensor("ident", [HD, HD], BF16, kind="ExternalInput")
    zpad = nc.dram_tensor("zpad", [128, PN], F32R, kind="ExternalInput")

    with TileContext(nc) as tc:
        _emit(nc, tc, xin, outx, diags, wq, wkv, wp, w1, w2, pb, b1, b2, ident, zpad)
    nc.compile()
    return nc


def _emit(nc, tc, xin, outx, diags, wq, wkv, wp, w1, w2, pb, b1, b2, ident, zpad):
    with contextlib.ExitStack() as ctx:
        consts = ctx.enter_context(tc.tile_pool(name="consts", bufs=1))
        carr = ctx.enter_context(tc.tile_pool(name="carr", bufs=2))
        carr1 = ctx.enter_context(tc.tile_pool(name="carr1", bufs=1))
        acts = ctx.enter_context(tc.tile_pool(name="acts", bufs=1))
        xbp = ctx.enter_context(tc.tile_pool(name="xbp", bufs=2))
        smalls = ctx.enter_context(tc.tile_pool(name="smalls", bufs=4))
        mlpp = ctx.enter_context(tc.tile_pool(name="mlpp", bufs=2))
        ps_big = ctx.enter_context(tc.tile_pool(name="ps_big", bufs=3,
                                                space="PSUM"))
        ps_row = ctx.enter_context(tc.tile_pool(name="ps_row", bufs=1,
                                                space="PSUM"))
        ps_sm = ctx.enter_context(tc.tile_pool(name="ps_sm", bufs=1,
                                               space="PSUM"))

        # ---- resident constants ----
        sdg = consts.tile([128, 2 * 9 * CT * 128], F32R)
        swq = consts.tile([128, CT * C], BF16)
        swkv = consts.tile([128, CT * 2 * C], BF16)
        swp = consts.tile([128, CT * C], BF16)
        sw1 = consts.tile([128, CT * MLP], BF16)
        sw2 = consts.tile([128, MT * C], BF16)
        spb = consts.tile([128, CT], F32)
        sb1 = consts.tile([128, MT], F32)
        sb2 = consts.tile([128, CT], F32)
        sident = consts.tile([HD, HD], BF16)
        zt = consts.tile([128, PN], F32R)
        nc.sync.dma_start(out=zt[:], in_=zpad[:])
        for dst, src in ((sdg, diags), (swq, wq), (swkv, wkv), (swp, wp),
                         (sw1, w1), (sw2, w2), (spb, pb), (sb1, b1),
                         (sb2, b2), (sident, ident)):
            nc.sync.dma_start(out=dst[:], in_=src[:])
        ones_r = consts.tile([128, 1], F32R)
        ones_f = consts.tile([128, 1], F32)
        ones_b = consts.tile([128, 1], BF16)
        eps_t = consts.tile([1, 1], F32)
        nc.vector.memset(ones_f[:], 1.0)
        nc.vector.tensor_copy(ones_r[:], ones_f[:])  # f32r memset is illegal
        nc.vector.memset(ones_b[:], 1.0)
        nc.vector.memset(eps_t[:], EPS)

        def dgv(cv, j, ct):
            b = (cv * 9 + j) * CT + ct
            return sdg[:, b * 128:(b + 1) * 128]

        def conv(src_pad, cv, dst):
            """dst = src + dwconv3x3(src); center tap has +1 folded in.
            src_pad: [CT][128, PH, PW] zero-padded; dst: [CT][128, N]."""
            for ct in range(CT):
                for ch in range(CH):
                    ps = ps_big.tile([128, CHW], F32, tag="ps", name="ps")
                    j = 0
                    for dh in (-1, 0, 1):
                        for dw in (-1, 0, 1):
                            nc.tensor.matmul(
                                ps[:], dgv(cv, j, ct),
                                _iview(src_pad[ct], ch, dh, dw),
                                start=(j == 0), stop=(j == 8))
                            j += 1
                    nc.scalar.copy(out=dst[ct][:, ch * CHW:(ch + 1) * CHW],
                                   in_=ps[:])

        def layernorm(src, xn, acc):
            """xn (bf16) = (src - mean_c) * rsqrt(var_c + eps); src [CT][128,N]."""
            sq = [acts.tile([128, N], BF16, tag=f"sq{t}", name=f"sq{t}")
                  for t in range(CT)]
            for t in range(CT):
                nc.scalar.activation(out=sq[t][:], in_=src[t][:],
                                     func=AF.Square)
            arow = smalls.tile([1, N], BF16, tag="arow", name="arow")
            brow = smalls.tile([1, N], BF16, tag="brow", name="brow")
            for ch in range(CH):
                sl = slice(ch * CHW, (ch + 1) * CHW)
                ps_s = ps_row.tile([1, CHW], F32, tag="psum_s", name="psum_s")
                ps_q = ps_row.tile([1, CHW], F32, tag="psum_q", name="psum_q")
                for t in range(CT):
                    nc.tensor.matmul(ps_s[:], ones_r[:], src[t][:, sl],
                                     start=(t == 0), stop=(t == CT - 1))
                for t in range(CT):
                    nc.tensor.matmul(ps_q[:], ones_b[:], sq[t][:, sl],
                                     start=(t == 0), stop=(t == CT - 1))
                u = smalls.tile([1, CHW], F32, tag="u", name="u")
                nc.scalar.activation(out=u[:], in_=ps_s[:], func=AF.Square,
                                     scale=float(1.0 / C))          # mean^2
                v = smalls.tile([1, CHW], F32, tag="v", name="v")
                nc.vector.scalar_tensor_tensor(
                    out=v[:], in0=ps_q[:], scalar=float(1.0 / C), in1=u[:],
                    op0=ALU.mult, op1=ALU.subtract)                 # var
                lv = smalls.tile([1, CHW], F32, tag="lv", name="lv")
                i_ln = nc.scalar.activation(out=lv[:], in_=v[:], func=AF.Ln,
                                            bias=eps_t[:])
                acc.append(i_ln)
                acc.append(nc.scalar.activation(
                    out=arow[:, sl], in_=lv[:], func=AF.Exp, scale=-0.5))
                nc.vector.scalar_tensor_tensor(
                    out=brow[:, sl], in0=ps_s[:], scalar=float(-1.0 / C),
                    in1=arow[:, sl], op0=ALU.mult, op1=ALU.mult)    # -mean*rstd
            ab = acts.tile([128, N], BF16, tag="ab", name="ab")
            bb = acts.tile([128, N], BF16, tag="bb", name="bb")
            nc.gpsimd.partition_broadcast(ab[:], arow[:])
            nc.gpsimd.partition_broadcast(bb[:], brow[:])
            for t in range(CT):
                t1 = xbp.tile([128, N], BF16, tag="lnt1", name="lnt1")
                nc.vector.tensor_tensor(out=t1[:], in0=src[t][:], in1=ab[:],
                                        op=ALU.mult)
                nc.vector.tensor_tensor(out=xn[t][:], in0=t1[:], in1=bb[:],
                                        op=ALU.add)

        def attention(xn, x2, carrier1, acc):
            # q channel-major [CT][128, N]
            q = [acts.tile([128, N], BF16, tag=f"q{m}", name=f"q{m}")
                 for m in range(CT)]
            for m in range(CT):
                for ch in range(CH):
                    sl = slice(ch * CHW, (ch + 1) * CHW)
                    ps = ps_big.tile([128, CHW], F32, tag="ps", name="ps")
                    for t in range(CT):
                        nc.tensor.matmul(
                            ps[:], swq[:, t * C + m * 128: t * C + (m + 1) * 128],
                            xn[t][:, sl], start=(t == 0), stop=(t == CT - 1))
                    nc.vector.tensor_copy(q[m][:, sl], ps[:])
            # k|v token-major [TOKT][112, 768]
            kv = [acts.tile([TOKW, 2 * C], BF16, tag=f"kv{i}", name=f"kv{i}")
                  for i in range(TOKT)]
            for i in range(TOKT):
                tok = slice(i * TOKW, (i + 1) * TOKW)
                for nch in range(2):
                    ps = ps_big.tile([TOKW, C], F32, tag="ps", name="ps")
                    for t in range(CT):
                        nc.tensor.matmul(
                            ps[:], xn[t][:, tok],
                            swkv[:, t * 2 * C + nch * C: t * 2 * C + (nch + 1) * C],
                            start=(t == 0), stop=(t == CT - 1))
                    nc.vector.tensor_copy(kv[i][:, nch * C:(nch + 1) * C],
                                          ps[:])
            # per-head gram -> exp(+rowsum) -> transpose -> block-diag ET
            etb = [acts.tile([128, C], BF16, tag=f"etb{t}", name=f"etb{t}")
                   for t in range(CT)]
            rz = [smalls.tile([128, 1], F32, tag=f"rz{t}", name=f"rz{t}")
                  for t in range(CT)]
            for t in range(CT):
                nc.vector.memset(etb[t][:], 0.0)
            for h in range(HEADS):
                psg = ps_sm.tile([HD, HD], F32, tag="gram", name="gram")
                for i in range(TOKT):
                    nc.tensor.matmul(
                        psg[:], kv[i][:, h * HD:(h + 1) * HD],
                        kv[i][:, C + h * HD:C + (h + 1) * HD],
                        start=(i == 0), stop=(i == TOKT - 1))
                e_s = smalls.tile([HD, HD], BF16, tag="e_s", name="e_s")
                z = smalls.tile([HD, 1], F32, tag="z", name="z")
                acc.append(nc.scalar.activation(out=e_s[:], in_=psg[:],
                                                func=AF.Exp, accum_out=z[:]))
                pst = ps_sm.tile([HD, HD], BF16, tag="et", name="et")
                nc.tensor.transpose(pst[:], e_s[:], sident[:])
                ets = smalls.tile([HD, HD], BF16, tag="ets", name="ets")
                nc.vector.tensor_copy(ets[:], pst[:])
                zr = smalls.tile([HD, 1], F32, tag="zr", name="zr")
                nc.vector.reciprocal(zr[:], z[:])
                r0 = h * HD
                # scatter via DMA (SWDGE): compute-op SBUF APs must be
                # 32-aligned in start partition; DMA is exempt.
                for (tt, lo, hi) in _row_pieces(r0, r0 + HD):
                    nc.sync.dma_start(
                        out=etb[tt][lo - 128 * tt:hi - 128 * tt, r0:r0 + HD],
                        in_=ets[lo - r0:hi - r0, :])
                    nc.sync.dma_start(
                        out=rz[tt][lo - 128 * tt:hi - 128 * tt, :],
                        in_=zr[lo - r0:hi - r0, :])
            # attn out (unnormalized) then 1/Z in the copy
            ao = [acts.tile([128, N], BF16, tag=f"ao{m}", name=f"ao{m}")
                  for m in range(CT)]
            for m in range(CT):
                for ch in range(CH):
                    sl = slice(ch * CHW, (ch + 1) * CHW)
                    ps = ps_big.tile([128, CHW], F32, tag="ps", name="ps")
                    for t in range(CT):
                        nc.tensor.matmul(
                            ps[:], etb[t][:, m * 128:(m + 1) * 128],
                            q[t][:, sl], start=(t == 0), stop=(t == CT - 1))
                    nc.vector.tensor_scalar_mul(ao[m][:, sl], ps[:], rz[m][:])
            # proj + bias + residual -> x2
            for m in range(CT):
                for ch in range(CH):
                    sl = slice(ch * CHW, (ch + 1) * CHW)
                    ps = ps_big.tile([128, CHW], F32, tag="ps", name="ps")
                    for t in range(CT):
                        nc.tensor.matmul(
                            ps[:], swp[:, t * C + m * 128: t * C + (m + 1) * 128],
                            ao[t][:, sl], start=(t == 0), stop=(t == CT - 1))
                    nc.vector.scalar_tensor_tensor(
                        out=_iview(x2[m], ch),
                        in0=ps[:].rearrange("p (a b) -> p a b", a=CHR),
                        scalar=spb[:, m:m + 1],
                        in1=carrier1[m][:, sl].rearrange("p (a b) -> p a b",
                                                         a=CHR),
                        op0=ALU.add, op1=ALU.add)

        def mlp(xn2, carrier2, ot, acc):
            for ch in range(CH):
                sl = slice(ch * CHW, (ch + 1) * CHW)
                h1 = [mlpp.tile([128, CHW], BF16, tag=f"h1_{m}", name=f"h1_{m}")
                      for m in range(MT)]
                for m in range(MT):
                    ps = ps_big.tile([128, CHW], F32, tag="ps", name="ps")
                    for t in range(CT):
                        nc.tensor.matmul(
                            ps[:],
                            sw1[:, t * MLP + m * 128: t * MLP + (m + 1) * 128],
                            xn2[t][:, sl], start=(t == 0), stop=(t == CT - 1))
                    acc.append(nc.scalar.activation(
                        out=h1[m][:], in_=ps[:], func=AF.Gelu,
                        bias=sb1[:, m:m + 1]))
                for m in range(CT):
                    ps = ps_big.tile([128, CHW], F32, tag="ps", name="ps")
                    for kt in range(MT):
                        nc.tensor.matmul(
                            ps[:], sw2[:, kt * C + m * 128: kt * C + (m + 1) * 128],
                            h1[kt][:], start=(kt == 0), stop=(kt == MT - 1))
                    nc.vector.scalar_tensor_tensor(
                        out=ot[m][:, sl], in0=ps[:], scalar=sb2[:, m:m + 1],
                        in1=carrier2[m][:, sl], op0=ALU.add, op1=ALU.add)

        prev_gelus = None
        for e in range(BE):
            c0 = [carr.tile([128, PH, PW], F32R, tag=f"pad{t}", name=f"pad{t}")
                  for t in range(CT)]
            for t in range(CT):
                nc.sync.dma_start(
                    out=c0[t][:],
                    in_=xin[e, t * 128:(t + 1) * 128, :].rearrange(
                        "p (a b) -> p a b", a=PH))
            carrier1 = [carr1.tile([128, N], F32R, tag=f"c1_{t}",
                                   name=f"c1_{t}") for t in range(CT)]
            conv(c0, 0, carrier1)
            # ACT table-set ordering: all exp/ln of this elem come after the
            # previous elem's gelus (keeps set switches at 2 per element).
            a_ops = []
            g_ops = []
            xn1 = [acts.tile([128, N], BF16, tag=f"xn1_{t}", name=f"xn1_{t}")
                   for t in range(CT)]
            layernorm(carrier1, xn1, a_ops)
            x2 = [carr.tile([128, PH, PW], F32R, tag=f"pad{t}", name=f"pad{t}")
                  for t in range(CT)]
            for t in range(CT):
                nc.vector.tensor_copy(x2[t][:], zt[:].rearrange(
                    "p (a b) -> p a b", a=PH))
            attention(xn1, x2, carrier1, a_ops)
            carrier2 = [carr1.tile([128, N], F32R, tag=f"c2_{t}",
                                   name=f"c2_{t}") for t in range(CT)]
            conv(x2, 1, carrier2)
            xn2 = [acts.tile([128, N], BF16, tag=f"xn2_{t}", name=f"xn2_{t}")
                   for t in range(CT)]
            layernorm(carrier2, xn2, a_ops)
            ot = [carr1.tile([128, N], F32, tag=f"ot{t}", name=f"ot{t}")
                  for t in range(CT)]
            mlp(xn2, carrier2, ot, g_ops)
            if prev_gelus and int(os.environ.get('ACT_ORDER', '0')):
                # every gelu (set B) of the previous elem precedes every
                # exp/ln (set A) of this elem -> 2 table switches per elem
                for g in prev_gelus:
                    add_dep_helper(g.ins, a_ops[0].ins, sync=True,
                                   reason="ACT set ordering")
                for op in a_ops[1:]:
                    add_dep_helper(a_ops[0].ins, op.ins, sync=True,
                                   reason="ACT set ordering")
            prev_gelus = g_ops
            for t in range(CT):
                nc.sync.dma_start(out=outx[e, t * 128:(t + 1) * 128, :],
                                  in_=ot[t][:])


# ======================= host side =======================

def _tobf(a):
    return np.asarray(a, np.float32).astype(ml_dtypes.bfloat16)


def _prep_weights(inputs):
    qkv_w = np.asarray(inputs["qkv_w"], np.float32)
    g1 = np.asarray(inputs["norm1_g"], np.float32)
    g2 = np.asarray(inputs["norm2_g"], np.float32)
    assert np.allclose(np.asarray(inputs["norm1_b"], np.float32), 0)
    assert np.allclose(np.asarray(inputs["norm2_b"], np.float32), 0)
    assert np.allclose(np.asarray(inputs["cpe0_b"], np.float32), 0)
    assert np.allclose(np.asarray(inputs["cpe1_b"], np.float32), 0)

    scale = HD ** (-0.5)
    wq_t = (qkv_w[0:C] * g1[None, :]).T.copy()              # [in C, out C]
    wk_t = (qkv_w[C:2 * C] * scale * g1[None, :]).T.copy()
    wv_t = (qkv_w[2 * C:3 * C] * g1[None, :]).T.copy()
    wkv_t = np.concatenate([wk_t, wv_t], axis=1)            # [C, 2C]
    wp_t = np.asarray(inputs["proj_w"], np.float32).T.copy()
    w1_t = (np.asarray(inputs["fc1_w"], np.float32) * g2[None, :]).T.copy()
    w2_t = np.asarray(inputs["fc2_w"], np.float32).T.copy()

    def kpack(wt):  # [Cin, F] -> [128, (Cin/128) * F] K-tile blocks
        kin, f = wt.shape
        nt = kin // 128
        return np.concatenate(
            [wt[t * 128:(t + 1) * 128, :] for t in range(nt)], axis=1).copy()

    def diag_pack():
        d = np.zeros((128, 2 * 9 * CT * 128), np.float32)
        for cv, wname in ((0, "cpe0_w"), (1, "cpe1_w")):
            w = np.asarray(inputs[wname], np.float32).reshape(C, 9)
            for j in range(9):
                wj = w[:, j].copy()
                if j == 4:
                    wj = wj + 1.0
                for t in range(CT):
                    blk = (cv * 9 + j) * CT + t
                    dd = d[:, blk * 128:(blk + 1) * 128]
                    np.fill_diagonal(dd, wj[t * 128:(t + 1) * 128])
        return d

    def colpack(b, nt):  # [nt*128] -> [128, nt]
        return np.asarray(b, np.float32).reshape(nt, 128).T.copy()

    return {
        "diags": diag_pack(),
        "wq": _tobf(kpack(wq_t)), "wkv": _tobf(kpack(wkv_t)),
        "wp": _tobf(kpack(wp_t)), "w1": _tobf(kpack(w1_t)),
        "w2": _tobf(kpack(w2_t)),
        "pb": colpack(inputs["proj_b"], CT),
        "b1": colpack(inputs["fc1_b"], MT),
        "b2": colpack(inputs["fc2_b"], CT),
        "ident": _tobf(np.eye(HD, dtype=np.float32)),
        "zpad": np.zeros((128, PN), np.float32),
    }


def _pad_x(xe):
    """[BE, N, C] -> channel-major zero-padded [BE, C, PN] float32."""
    xp = np.zeros((xe.shape[0], C, PH, PW), np.float32)
    xc = np.transpose(np.asarray(xe, np.float32), (0, 2, 1))
    xp[:, :, 1:H + 1, 1:W + 1] = xc.reshape(-1, C, H, W)
    return xp.reshape(-1, C, PN)


def kernel(**inputs):
    x = np.asarray(inputs["x"], np.float32)
    if "nc" not in _CACHE:
        _CACHE["nc"] = build_module()
    nc = _CACHE["nc"]
    wmap = _prep_weights(inputs)

    in_maps = []
    for core in range(NCORES):
        xe = x[core * BE:(core + 1) * BE]                   # [BE, N, C]
        m = {"xin": _pad_x(xe)}
        m.update(wmap)
        in_maps.append(m)

    trace = bool(int(os.environ.get("TRN_KERNEL_TRACE", "0")))
    res = bass_utils.run_bass_kernel_spmd(
        nc, in_maps, core_ids=list(range(NCORES)), trace=trace)
    if trace:
        _CACHE["exec_time_ns"] = res.exec_time_ns

    outp = np.empty((B, N, C), np.float32)
    for core in range(NCORES):
        oc = res.results[core]["outx"]
        outp[core * BE:(core + 1) * BE] = np.transpose(oc, (0, 2, 1))
    return outp


# revision 21
# speedup vs baseline: 1.1935x; 1.0087x over previous
"""ChannelBlock kernel for Trainium2 — 8 NeuronCores, data-parallel over batch.

Per batch elem (C=384, N=784=28x28, 8 heads, hd=48, mlp=1536):
  x1  = x + dwconv3x3(x, cpe0)
  x2  = x1 + proj(chan_attn(LN(x1))) + proj_b
  x3  = x2 + dwconv3x3(x2, cpe1)
  out = x3 + fc2(gelu(fc1(LN(x3)) + b1)) + b2

Device: channel-major carrier [C, N] (no spatial padding; conv boundary
handled by clipped access patterns), float32r on the carrier path (conv diag
matmuls, LN stats), bf16 on transformer matmuls.
kernel(**inputs) accepts the full unsharded inputs and returns full output.
"""

import contextlib
import os
import numpy as np
import ml_dtypes

import concourse.bacc as bacc
import concourse.bass as bass
from concourse import mybir, bass_utils
from concourse.tile import TileContext, add_dep_helper

F32 = mybir.dt.float32
F32R = mybir.dt.float32r
BF16 = mybir.dt.bfloat16
AF = mybir.ActivationFunctionType
ALU = mybir.AluOpType

B = 32
NCORES = 8
BE = B // NCORES
C = 384
H = W = 28
N = H * W
PW = W + 2            # 30
PH = H + 2            # 30
PN = PH * PW          # 900
HEADS = 8
HD = C // HEADS       # 48
MLP = 4 * C
EPS = 1e-5
CT = C // 128         # 3
MT = MLP // 128       # 12
CH = 2                # token chunks (of 14 spatial rows)
CHR = 14              # rows per chunk
CHW = N // CH         # 392
TOKT = 7
TOKW = N // TOKT      # 112

_CACHE = {}


def _iview(t, ch, dh=0, dw=0):
    """Interior chunk view of padded [128, PH, PW] tile at spatial shift."""
    h0 = CHR * ch + 1 + dh
    return t[:, h0:h0 + CHR, 1 + dw:29 + dw]


def _row_pieces(lo, hi):
    out = []
    t = lo // 128
    while lo < hi:
        top = min(hi, (t + 1) * 128)
        out.append((t, lo, top))
        lo = top
        t += 1
    return out


def build_module():
    nc = bacc.Bacc("TRN2", target_bir_lowering=False, debug=False,
                   num_devices=NCORES)
    xin = nc.dram_tensor("xin", [BE, C, PN], F32R, kind="ExternalInput")
    outx = nc.dram_tensor("outx", [BE, C, N], F32, kind="ExternalOutput")
    diags = nc.dram_tensor("diags", [128, 2 * 9 * CT * 128], BF16,
                           kind="ExternalInput")
    wq = nc.dram_tensor("wq", [128, CT * C], BF16, kind="ExternalInput")
    wkv = nc.dram_tensor("wkv", [128, CT * 2 * C], BF16, kind="ExternalInput")
    wp = nc.dram_tensor("wp", [128, CT * C], BF16, kind="ExternalInput")
    w1 = nc.dram_tensor("w1", [128, CT * MLP], BF16, kind="ExternalInput")
    w2 = nc.dram_tensor("w2", [128, MT * C], BF16, kind="ExternalInput")
    pb = nc.dram_tensor("pb", [128, CT], F32, kind="ExternalInput")
    b1 = nc.dram_tensor("b1", [128, MT], F32, kind="ExternalInput")
    b2 = nc.dram_tensor("b2", [128, CT], F32, kind="ExternalInput")
    ident = nc.dram_tensor("ident", [HD, HD], BF16, kind="ExternalInput")
    zpad = nc.dram_tensor("zpad", [128, PN], F32R, kind="ExternalInput")

    with TileContext(nc) as tc:
        _emit(nc, tc, xin, outx, diags, wq, wkv, wp, w1, w2, pb, b1, b2, ident, zpad)
    nc.compile()
    return nc


def _emit(nc, tc, xin, outx, diags, wq, wkv, wp, w1, w2, pb, b1, b2, ident, zpad):
    with contextlib.ExitStack() as ctx:
        consts = ctx.enter_context(tc.tile_pool(name="consts", bufs=1))
        carr = ctx.enter_context(tc.tile_pool(name="carr", bufs=2))
        carr1 = ctx.enter_context(tc.tile_pool(name="carr1", bufs=1))
        acts = ctx.enter_context(tc.tile_pool(name="acts", bufs=1))
        xbp = ctx.enter_context(tc.tile_pool(name="xbp", bufs=2))
        smalls = ctx.enter_context(tc.tile_pool(name="smalls", bufs=4))
        mlpp = ctx.enter_context(tc.tile_pool(name="mlpp", bufs=2))
        ps_conv = ctx.enter_context(tc.tile_pool(name="ps_conv", bufs=1,
                                                 space="PSUM"))
        ps_attn = ctx.enter_context(tc.tile_pool(name="ps_attn", bufs=2,
                                                 space="PSUM"))
        ps_mlp = ctx.enter_context(tc.tile_pool(name="ps_mlp", bufs=2,
                                                space="PSUM"))
        ps_row = ctx.enter_context(tc.tile_pool(name="ps_row", bufs=2,
                                                space="PSUM"))
        ps_sm = ctx.enter_context(tc.tile_pool(name="ps_sm", bufs=1,
                                               space="PSUM"))

        # ---- resident constants ----
        sdg = consts.tile([128, 2 * 9 * CT * 128], BF16)
        swq = consts.tile([128, CT * C], BF16)
        swkv = consts.tile([128, CT * 2 * C], BF16)
        swp = consts.tile([128, CT * C], BF16)
        sw1 = consts.tile([128, CT * MLP], BF16)
        sw2 = consts.tile([128, MT * C], BF16)
        spb = consts.tile([128, CT], F32)
        sb1 = consts.tile([128, MT], F32)
        sb2 = consts.tile([128, CT], F32)
        sident = consts.tile([HD, HD], BF16)
        zt = consts.tile([128, PN], F32R)
        nc.sync.dma_start(out=zt[:], in_=zpad[:])
        for dst, src in ((sdg, diags), (swq, wq), (swkv, wkv), (swp, wp),
                         (sw1, w1), (sw2, w2), (spb, pb), (sb1, b1),
                         (sb2, b2), (sident, ident)):
            nc.sync.dma_start(out=dst[:], in_=src[:])
        ones_r = consts.tile([128, 1], F32R)
        ones_f = consts.tile([128, 1], F32)
        ones_b = consts.tile([128, 1], BF16)
        eps_t = consts.tile([1, 1], F32)
        nc.vector.memset(ones_f[:], 1.0)
        nc.vector.tensor_copy(ones_r[:], ones_f[:])  # f32r memset is illegal
        nc.vector.memset(ones_b[:], 1.0)
        nc.vector.memset(eps_t[:], EPS)

        def dgv(cv, j, ct):
            b = (cv * 9 + j) * CT + ct
            return sdg[:, b * 128:(b + 1) * 128]

        def conv(src_pad, cv, dst):
            """dst = src + dwconv3x3(src), bf16 taps on a shadow cast of the
            padded carrier; exact f32r residual added in the psum->sbuf STT.
            src_pad: [CT][128, PH, PW] f32r zero-padded; dst: [CT][128, N]."""
            shad = [xbp.tile([128, PH, PW], BF16, tag=f"shad{t}",
                             name=f"shad{t}") for t in range(CT)]
            for t in range(CT):
                nc.vector.tensor_copy(shad[t][:], src_pad[t][:])
            for ct in range(CT):
                for ch in range(CH):
                    ps = ps_conv.tile([128, CHW], F32, tag="psc", name="psc")
                    j = 0
                    for dh in (-1, 0, 1):
                        for dw in (-1, 0, 1):
                            nc.tensor.matmul(
                                ps[:], dgv(cv, j, ct),
                                _iview(shad[ct], ch, dh, dw),
                                start=(j == 0), stop=(j == 8))
                            j += 1
                    nc.vector.scalar_tensor_tensor(
                        out=dst[ct][:, ch * CHW:(ch + 1) * CHW].rearrange(
                            "p (a b) -> p a b", a=CHR),
                        in0=ps[:].rearrange("p (a b) -> p a b", a=CHR),
                        scalar=0.0, in1=_iview(src_pad[ct], ch),
                        op0=ALU.add, op1=ALU.add)

        def layernorm(src, xn, acc, ln_heads):
            """xn (bf16) = (src - mean_c) * rsqrt(var_c + eps); src [CT][128,N]."""
            sq = [acts.tile([128, N], BF16, tag=f"sq{t}", name=f"sq{t}")
                  for t in range(CT)]
            for t in range(CT):
                nc.scalar.activation(out=sq[t][:], in_=src[t][:],
                                     func=AF.Square)
            arow = smalls.tile([1, N], BF16, tag="arow", name="arow")
            brow = smalls.tile([1, N], BF16, tag="brow", name="brow")
            for ch in range(CH):
                sl = slice(ch * CHW, (ch + 1) * CHW)
                ps_s = ps_row.tile([1, CHW], F32, tag="psr", name="psr")
                ps_q = ps_row.tile([1, CHW], F32, tag="psr", name="psr")
                for t in range(CT):
                    nc.tensor.matmul(ps_s[:], ones_r[:], src[t][:, sl],
                                     start=(t == 0), stop=(t == CT - 1))
                for t in range(CT):
                    nc.tensor.matmul(ps_q[:], ones_b[:], sq[t][:, sl],
                                     start=(t == 0), stop=(t == CT - 1))
                u = smalls.tile([1, CHW], F32, tag="u", name="u")
                nc.scalar.activation(out=u[:], in_=ps_s[:], func=AF.Square,
                                     scale=float(1.0 / C))          # mean^2
                v = smalls.tile([1, CHW], F32, tag="v", name="v")
                nc.vector.scalar_tensor_tensor(
                    out=v[:], in0=ps_q[:], scalar=float(1.0 / C), in1=u[:],
                    op0=ALU.mult, op1=ALU.subtract)                 # var
                lv = smalls.tile([1, CHW], F32, tag="lv", name="lv")
                i_ln = nc.scalar.activation(out=lv[:], in_=v[:], func=AF.Ln,
                                            bias=eps_t[:])
                acc.append(i_ln)
                ln_heads.append(i_ln)
                acc.append(nc.scalar.activation(
                    out=arow[:, sl], in_=lv[:], func=AF.Exp, scale=-0.5))
                nc.vector.scalar_tensor_tensor(
                    out=brow[:, sl], in0=ps_s[:], scalar=float(-1.0 / C),
                    in1=arow[:, sl], op0=ALU.mult, op1=ALU.mult)    # -mean*rstd
            ab = acts.tile([128, N], BF16, tag="ab", name="ab")
            bb = acts.tile([128, N], BF16, tag="bb", name="bb")
            nc.gpsimd.partition_broadcast(ab[:], arow[:])
            nc.gpsimd.partition_broadcast(bb[:], brow[:])
            for t in range(CT):
                t1 = xbp.tile([128, N], BF16, tag="lnt1", name="lnt1")
                nc.vector.tensor_tensor(out=t1[:], in0=src[t][:], in1=ab[:],
                                        op=ALU.mult)
                nc.vector.tensor_tensor(out=xn[t][:], in0=t1[:], in1=bb[:],
                                        op=ALU.add)

        def attention(xn, x2, carrier1, acc):
            # q channel-major [CT][128, N]
            q = [acts.tile([128, N], BF16, tag=f"q{m}", name=f"q{m}")
                 for m in range(CT)]
            for m in range(CT):
                for ch in range(CH):
                    sl = slice(ch * CHW, (ch + 1) * CHW)
                    ps = ps_attn.tile([128, CHW], F32, tag="psa", name="psa")
                    for t in range(CT):
                        nc.tensor.matmul(
                            ps[:], swq[:, t * C + m * 128: t * C + (m + 1) * 128],
                            xn[t][:, sl], start=(t == 0), stop=(t == CT - 1))
                    nc.vector.tensor_copy(q[m][:, sl], ps[:])
            # k|v token-major [TOKT][112, 768]
            kv = [acts.tile([TOKW, 2 * C], BF16, tag=f"kv{i}", name=f"kv{i}")
                  for i in range(TOKT)]
            for i in range(TOKT):
                tok = slice(i * TOKW, (i + 1) * TOKW)
                for nch in range(2):
                    ps = ps_attn.tile([TOKW, C], F32, tag="psa", name="psa")
                    for t in range(CT):
                        nc.tensor.matmul(
                            ps[:], xn[t][:, tok],
                            swkv[:, t * 2 * C + nch * C: t * 2 * C + (nch + 1) * C],
                            start=(t == 0), stop=(t == CT - 1))
                    nc.vector.tensor_copy(kv[i][:, nch * C:(nch + 1) * C],
                                          ps[:])
            # per-head gram -> exp(+rowsum) -> transpose -> block-diag ET
            etb = [acts.tile([128, C], BF16, tag=f"etb{t}", name=f"etb{t}")
                   for t in range(CT)]
            rz = [smalls.tile([128, 1], F32, tag=f"rz{t}", name=f"rz{t}")
                  for t in range(CT)]
            for t in range(CT):
                nc.vector.memset(etb[t][:], 0.0)
            for h in range(HEADS):
                psg = ps_sm.tile([HD, HD], F32, tag="pss", name="pss")
                for i in range(TOKT):
                    nc.tensor.matmul(
                        psg[:], kv[i][:, h * HD:(h + 1) * HD],
                        kv[i][:, C + h * HD:C + (h + 1) * HD],
                        start=(i == 0), stop=(i == TOKT - 1))
                e_s = smalls.tile([HD, HD], BF16, tag="e_s", name="e_s")
                z = smalls.tile([HD, 1], F32, tag="z", name="z")
                acc.append(nc.scalar.activation(out=e_s[:], in_=psg[:],
                                                func=AF.Exp, accum_out=z[:]))
                pst = ps_sm.tile([HD, HD], BF16, tag="pss", name="pss")
                nc.tensor.transpose(pst[:], e_s[:], sident[:])
                ets = smalls.tile([HD, HD], BF16, tag="ets", name="ets")
                nc.vector.tensor_copy(ets[:], pst[:])
                zr = smalls.tile([HD, 1], F32, tag="zr", name="zr")
                nc.vector.reciprocal(zr[:], z[:])
                r0 = h * HD
                # scatter via DMA (SWDGE): compute-op SBUF APs must be
                # 32-aligned in start partition; DMA is exempt.
                for (tt, lo, hi) in _row_pieces(r0, r0 + HD):
                    nc.sync.dma_start(
                        out=etb[tt][lo - 128 * tt:hi - 128 * tt, r0:r0 + HD],
                        in_=ets[lo - r0:hi - r0, :])
                    nc.sync.dma_start(
                        out=rz[tt][lo - 128 * tt:hi - 128 * tt, :],
                        in_=zr[lo - r0:hi - r0, :])
            # attn out (unnormalized) then 1/Z in the copy
            ao = [acts.tile([128, N], BF16, tag=f"ao{m}", name=f"ao{m}")
                  for m in range(CT)]
            for m in range(CT):
                for ch in range(CH):
                    sl = slice(ch * CHW, (ch + 1) * CHW)
                    ps = ps_attn.tile([128, CHW], F32, tag="psa", name="psa")
                    for t in range(CT):
                        nc.tensor.matmul(
                            ps[:], etb[t][:, m * 128:(m + 1) * 128],
                            q[t][:, sl], start=(t == 0), stop=(t == CT - 1))
                    nc.vector.tensor_scalar_mul(ao[m][:, sl], ps[:], rz[m][:])
            # proj + bias + residual -> x2
            for m in range(CT):
                for ch in range(CH):
                    sl = slice(ch * CHW, (ch + 1) * CHW)
                    ps = ps_attn.tile([128, CHW], F32, tag="psa", name="psa")
                    for t in range(CT):
                        nc.tensor.matmul(
                            ps[:], swp[:, t * C + m * 128: t * C + (m + 1) * 128],
                            ao[t][:, sl], start=(t == 0), stop=(t == CT - 1))
                    nc.vector.scalar_tensor_tensor(
                        out=_iview(x2[m], ch),
                        in0=ps[:].rearrange("p (a b) -> p a b", a=CHR),
                        scalar=spb[:, m:m + 1],
                        in1=carrier1[m][:, sl].rearrange("p (a b) -> p a b",
                                                         a=CHR),
                        op0=ALU.add, op1=ALU.add)

        def mlp(xn2, carrier2, ot, acc, fc1_last):
            for ch in range(CH):
                sl = slice(ch * CHW, (ch + 1) * CHW)
                h1 = [mlpp.tile([128, CHW], BF16, tag=f"h1_{m}", name=f"h1_{m}")
                      for m in range(MT)]
                for m in range(MT):
                    ps = ps_mlp.tile([128, CHW], F32, tag="psm", name="psm")
                    for t in range(CT):
                        mm = nc.tensor.matmul(
                            ps[:],
                            sw1[:, t * MLP + m * 128: t * MLP + (m + 1) * 128],
                            xn2[t][:, sl], start=(t == 0), stop=(t == CT - 1))
                    acc.append(nc.scalar.activation(
                        out=h1[m][:], in_=ps[:], func=AF.Gelu,
                        bias=sb1[:, m:m + 1]))
                    fc1_last[0] = mm
                for m in range(CT):
                    ps = ps_mlp.tile([128, CHW], F32, tag="psm", name="psm")
                    for kt in range(MT):
                        nc.tensor.matmul(
                            ps[:], sw2[:, kt * C + m * 128: kt * C + (m + 1) * 128],
                            h1[kt][:], start=(kt == 0), stop=(kt == MT - 1))
                    nc.vector.scalar_tensor_tensor(
                        out=ot[m][:, sl], in0=ps[:], scalar=sb2[:, m:m + 1],
                        in1=carrier2[m][:, sl], op0=ALU.add, op1=ALU.add)

        prev_anchor = None
        for e in range(BE):
            c0 = [carr.tile([128, PH, PW], F32R, tag=f"pad{t}", name=f"pad{t}")
                  for t in range(CT)]
            for t in range(CT):
                nc.sync.dma_start(
                    out=c0[t][:],
                    in_=xin[e, t * 128:(t + 1) * 128, :].rearrange(
                        "p (a b) -> p a b", a=PH))
            carrier1 = [carr1.tile([128, N], F32R, tag=f"c1_{t}",
                                   name=f"c1_{t}") for t in range(CT)]
            conv(c0, 0, carrier1)
            # ACT table-set ordering: all exp/ln of this elem come after the
            # previous elem's gelus (keeps set switches at 2 per element).
            a_ops = []
            g_ops = []
            xn1 = [acts.tile([128, N], BF16, tag=f"xn1_{t}", name=f"xn1_{t}")
                   for t in range(CT)]
            ln1_heads = []
            layernorm(carrier1, xn1, a_ops, ln1_heads)
            if prev_anchor is not None and int(os.environ.get('ACT_ORDER', '1')):
                for op in ln1_heads:
                    add_dep_helper(prev_anchor.ins, op.ins, sync=True,
                                   reason="ACT set ordering")
            x2 = [carr.tile([128, PH, PW], F32R, tag=f"pad{t}", name=f"pad{t}")
                  for t in range(CT)]
            for t in range(CT):
                nc.vector.tensor_copy(x2[t][:], zt[:].rearrange(
                    "p (a b) -> p a b", a=PH))
            attention(xn1, x2, carrier1, a_ops)
            carrier2 = [carr1.tile([128, N], F32R, tag=f"c2_{t}",
                                   name=f"c2_{t}") for t in range(CT)]
            conv(x2, 1, carrier2)
            xn2 = [acts.tile([128, N], BF16, tag=f"xn2_{t}", name=f"xn2_{t}")
                   for t in range(CT)]
            layernorm(carrier2, xn2, a_ops, [])
            ot = [carr1.tile([128, N], F32, tag=f"ot{t}", name=f"ot{t}")
                  for t in range(CT)]
            fc1_last = [None]
            mlp(xn2, carrier2, ot, g_ops, fc1_last)
            prev_fc1_last = fc1_last[0]
            prev_anchor = prev_fc1_last
            for t in range(CT):
                nc.sync.dma_start(out=outx[e, t * 128:(t + 1) * 128, :],
                                  in_=ot[t][:])


# ======================= host side =======================

def _tobf(a):
    return np.asarray(a, np.float32).astype(ml_dtypes.bfloat16)


def _prep_weights(inputs):
    qkv_w = np.asarray(inputs["qkv_w"], np.float32)
    g1 = np.asarray(inputs["norm1_g"], np.float32)
    g2 = np.asarray(inputs["norm2_g"], np.float32)
    assert np.allclose(np.asarray(inputs["norm1_b"], np.float32), 0)
    assert np.allclose(np.asarray(inputs["norm2_b"], np.float32), 0)
    assert np.allclose(np.asarray(inputs["cpe0_b"], np.float32), 0)
    assert np.allclose(np.asarray(inputs["cpe1_b"], np.float32), 0)

    scale = HD ** (-0.5)
    wq_t = (qkv_w[0:C] * g1[None, :]).T.copy()              # [in C, out C]
    wk_t = (qkv_w[C:2 * C] * scale * g1[None, :]).T.copy()
    wv_t = (qkv_w[2 * C:3 * C] * g1[None, :]).T.copy()
    wkv_t = np.concatenate([wk_t, wv_t], axis=1)            # [C, 2C]
    wp_t = np.asarray(inputs["proj_w"], np.float32).T.copy()
    w1_t = (np.asarray(inputs["fc1_w"], np.float32) * g2[None, :]).T.copy()
    w2_t = np.asarray(inputs["fc2_w"], np.float32).T.copy()

    def kpack(wt):  # [Cin, F] -> [128, (Cin/128) * F] K-tile blocks
        kin, f = wt.shape
        nt = kin // 128
        return np.concatenate(
            [wt[t * 128:(t + 1) * 128, :] for t in range(nt)], axis=1).copy()

    def diag_pack():
        d = np.zeros((128, 2 * 9 * CT * 128), np.float32)
        for cv, wname in ((0, "cpe0_w"), (1, "cpe1_w")):
            w = np.asarray(inputs[wname], np.float32).reshape(C, 9)
            for j in range(9):
                wj = w[:, j].copy()
                for t in range(CT):
                    blk = (cv * 9 + j) * CT + t
                    dd = d[:, blk * 128:(blk + 1) * 128]
                    np.fill_diagonal(dd, wj[t * 128:(t + 1) * 128])
        return d

    def colpack(b, nt):  # [nt*128] -> [128, nt]
        return np.asarray(b, np.float32).reshape(nt, 128).T.copy()

    return {
        "diags": _tobf(diag_pack()),
        "wq": _tobf(kpack(wq_t)), "wkv": _tobf(kpack(wkv_t)),
        "wp": _tobf(kpack(wp_t)), "w1": _tobf(kpack(w1_t)),
        "w2": _tobf(kpack(w2_t)),
        "pb": colpack(inputs["proj_b"], CT),
        "b1": colpack(inputs["fc1_b"], MT),
        "b2": colpack(inputs["fc2_b"], CT),
        "ident": _tobf(np.eye(HD, dtype=np.float32)),
        "zpad": np.zeros((128, PN), np.float32),
    }


def _pad_x(xe):
    """[BE, N, C] -> channel-major zero-padded [BE, C, PN] float32."""
    xp = np.zeros((xe.shape[0], C, PH, PW), np.float32)
    xc = np.transpose(np.asarray(xe, np.float32), (0, 2, 1))
    xp[:, :, 1:H + 1, 1:W + 1] = xc.reshape(-1, C, H, W)
    return xp.reshape(-1, C, PN)


def kernel(**inputs):
    x = np.asarray(inputs["x"], np.float32)
    if "nc" not in _CACHE:
        _CACHE["nc"] = build_module()
    nc = _CACHE["nc"]
    wmap = _prep_weights(inputs)

    in_maps = []
    for core in range(NCORES):
        xe = x[core * BE:(core + 1) * BE]                   # [BE, N, C]
        m = {"xin": _pad_x(xe)}
        m.update(wmap)
        in_maps.append(m)

    trace = bool(int(os.environ.get("TRN_KERNEL_TRACE", "0")))
    res = bass_utils.run_bass_kernel_spmd(
        nc, in_maps, core_ids=list(range(NCORES)), trace=trace)
    if trace:
        _CACHE["exec_time_ns"] = res.exec_time_ns

    outp = np.empty((B, N, C), np.float32)
    for core in range(NCORES):
        oc = res.results[core]["outx"]
        outp[core * BE:(core + 1) * BE] = np.transpose(oc, (0, 2, 1))
    return outp
